# revision 24
# baseline (speedup 1.0000x reference)
"""Trainium2 Bass kernel for AlphaFold-style gated MSA attention.

Reference computation (per batch b=1, per MSA row n of 64):
    q = (q_x @ wq) / sqrt(32);  k = k_x @ wk;  v = v_x @ wv      (heads: 8 x 32)
    a = softmax(q k^T + bias_mask[n,k] + bias_pair[h,q,k])
    o = (a @ v) * sigmoid(q_x @ wg + bg)
    out = o @ wo + bo

Distribution: data-parallel over the 64 MSA rows -> 8 rows per NeuronCore.

Per-core schedule (per row n):
  1. Host ships q_x/k_x/v_x pre-transposed [C, seq] in bf16 (PE runs
     bf16 at 1 cycle/row vs fp32 HIGH mode's ~2.8).
  2. Projections producing qT/kT (bf16), gateT (f32) and v (bf16).
  3. S^T = k_h q_h^T per head/key-chunk (bf16, K=32). bias_pair is added
     in-PSUM by bf16 identity matmuls on the PE: keeping the whole
     S->exp chain on PE->ACT (whose per-chunk rates are matched at
     ~1us) avoids the cross-engine convoys that stalled the PE when
     the bias rode DVE/GPSIMD. bias_mask folds into the ACT exp as a
     per-partition bias (S^T layout puts k on partitions). Softmax
     max-subtraction is skipped: logits are O(5), far from overflow.
  4. o^T_h = [v_h | 1]^T @ E_h  (M=33: row 32 accumulates the softmax
     denominator for free). E is bf16 straight out of the ACT exp.
     AV matmul groups of the previous head group are interleaved into
     the next S phase to fill the PE stalls where a chunk's PSUM
     recycle waits on the exp drain; the last row interleaves its own
     hg1 AV (pr-outer chunk order) to shrink the epilogue pipe-drain.
  5. Normalize by the broadcast reciprocal denominator (collected into a
     [128, 32] layout so DVE reciprocal is 256 cycles, not 4096), gate
     with sigmoid (via tanh; the 0.5 is folded into wo host-side),
     output-project in bf16, add bo. The gate/normalize chain runs
     split across GPSIMD and DVE, with broadcast DMAs issued from the
     GPSIMD software-DGE queue and den-collect DMAs alternating between
     the ACT and Sync queues — the Sync queue's ~630ns-per-DMA issue
     cost was itself a serializer at 32 DMAs/row.
"""

import math
import os
import sys

for _p in ("/opt/trn_rl_repo", "/root/.axon_site/_ro/trn_rl_repo"):
    if os.path.isdir(_p) and _p not in sys.path:
        sys.path.append(_p)

import numpy as np
import ml_dtypes

import bass_rust
import concourse.bass as bass
import concourse.mybir as mybir
import concourse.tile as tile
from concourse.bass_utils import run_bass_kernel_spmd
from concourse.masks import make_identity
from concourse.tile import ScopedClock

f32 = mybir.dt.float32
bf16 = mybir.dt.bfloat16
np_bf16 = ml_dtypes.bfloat16

N_CORES = 8
NL = 8        # MSA rows per core (64 / 8)
SEQ = 512     # q and k sequence length
C = 256       # channel dim of q_x/k_x/v_x and the output
HID = 256     # heads * c_hidden
H = 8         # heads
CH = 32       # c_hidden per head
P = 128
CC = C // P   # 2 contraction chunks for projections
HC = HID // P  # 2 hidden chunks
KC = SEQ // P  # 4 key chunks
QC = SEQ // P  # 4 query chunks
HG = 2        # head groups of 4

# bias_pair application engine per (hg, pr, kc):
#   "pe"  — bf16 identity matmul accumulated in PSUM (Tensor engine)
#   "dve" — tensor add on DVE
#   "gp"  — multiplicative exp(bias) on GPSIMD (host ships exp'd bias
#           for those heads)
def _bias_engine(hg, pr, kc):
    # All bias_pair chunks ride the PE as bf16 identity matmuls: the
    # exp chain then depends only on PE->ACT, whose per-chunk rates are
    # matched (~1us), so no cross-engine convoy can stall the PE.
    return "pe"

GP_HEADS = ()  # no multiplicative-bias heads


class _TileContextSplitWaits(tile.TileContext):
    """This container's walrus supports ONE sync-wait per instruction (the
    TRN2 EVENTS struct has a single wait slot and this build refuses to
    expand multi-wait instructions). Tile attaches several waits to one
    instruction; split the extras onto same-engine NOPs emitted just before
    it — the engine queue is in-order, so this is semantically identical."""

    def _add_instruction(self, inst):
        si = inst.sync_info
        if (
            si is not None
            and len(si.on_wait) > 1
            and inst.engine != mybir.EngineType.Unassigned
        ):
            waits = list(si.on_wait)
            for w in waits[:-1]:
                nop = mybir.InstNoOp(
                    name=self.nc.get_next_instruction_name(),
                    sync_info=mybir.SyncInfo(on_wait=[w], on_update=[]),
                    bass_nofuse=True,
                    engine=inst.engine,
                )
                super()._add_instruction(nop)
            inst.sync_info = mybir.SyncInfo(
                on_wait=waits[-1:], on_update=list(si.on_update)
            )
        super()._add_instruction(inst)

    def _drain_and_barrier(self, tick_clock, wait_clock):
        nc = self.nc
        drain_inst = nc.sync.drain()
        wait_clock.add_sem_waits(
            drain_inst.ins, ScopedClock({None: tick_clock.global_clock})
        )
        si = drain_inst.ins.sync_info
        if si is not None and len(si.on_wait) > 1:
            waits = list(si.on_wait)
            updates = list(si.on_update)
            drain_inst.ins.sync_info = bass_rust.SyncInfo(
                on_wait=waits[:1], on_update=[]
            )
            for i, w in enumerate(waits[1:]):
                upd = updates if i == len(waits) - 2 else []
                nop = nc.sync.nop()
                nop.ins.sync_info = bass_rust.SyncInfo(on_wait=[w], on_update=upd)
        nc.all_engine_barrier()
        assert self.sems is not None
        popped = nc._tile_sem_poison_stack.pop()
        assert popped is self._sem_poison
        nc.clear_and_free_semaphores(list(self.sems.allocated().values()))
        nc.all_engine_barrier()


def _build_nc():
    nc = bass.Bass(
        "TRN2", target_bir_lowering=False, debug=False, num_devices=N_CORES
    )
    qx = nc.dram_tensor("qx", [NL, C, SEQ], bf16, kind="ExternalInput").ap()
    kx = nc.dram_tensor("kx", [NL, C, SEQ], bf16, kind="ExternalInput").ap()
    vx = nc.dram_tensor("vx", [NL, C, SEQ], bf16, kind="ExternalInput").ap()
    bpt = nc.dram_tensor("bpt", [H, SEQ, SEQ], bf16, kind="ExternalInput").ap()
    bm = nc.dram_tensor("bm", [P, KC, NL], f32, kind="ExternalInput").ap()
    wq = nc.dram_tensor("wq", [C, HID], bf16, kind="ExternalInput").ap()
    wk = nc.dram_tensor("wk", [C, HID], bf16, kind="ExternalInput").ap()
    wv = nc.dram_tensor("wv", [C, HID], bf16, kind="ExternalInput").ap()
    wg = nc.dram_tensor("wg", [C, HID], bf16, kind="ExternalInput").ap()
    bgh = nc.dram_tensor("bgh", [P, HC], f32, kind="ExternalInput").ap()
    wo = nc.dram_tensor("wo", [HID, C], bf16, kind="ExternalInput").ap()
    bo_bc = nc.dram_tensor("bo_bc", [P, C], f32, kind="ExternalInput").ap()
    out = nc.dram_tensor("out", [NL, SEQ, C], f32, kind="ExternalOutput").ap()

    Exp = mybir.ActivationFunctionType.Exp
    Tanh = mybir.ActivationFunctionType.Tanh
    MULT = mybir.AluOpType.mult
    ADD = mybir.AluOpType.add

    with _TileContextSplitWaits(nc) as tc:
        with (
            tc.tile_pool(name="const", bufs=1) as const,
            tc.tile_pool(name="dram", bufs=2, space="DRAM") as drp,
        ):
            # --- constants ---------------------------------------------------
            w_sbs = {}
            # wg first: the gate projection is the first matmul of row 0.
            for name, w_ap in (("wg", wg), ("wq", wq), ("wk", wk), ("wv", wv)):
                w_sbs[name] = const.tile(
                    [P, CC, HID], bf16, tag=f"w_{name}", name=f"w_{name}"
                )
                nc.sync.dma_start(
                    out=w_sbs[name],
                    in_=w_ap.rearrange("(cc p) h -> p cc h", p=P),
                )
            wo_sb = const.tile([P, HC, C], bf16, tag="w_wo")
            nc.sync.dma_start(
                out=wo_sb, in_=wo.rearrange("(hc p) c -> p hc c", p=P)
            )
            bm_sb = const.tile([P, KC, NL], f32, tag="bm")
            nc.scalar.dma_start(out=bm_sb, in_=bm)
            bgh_sb = const.tile([P, HC], f32, tag="bgh")
            nc.scalar.dma_start(out=bgh_sb, in_=bgh)
            bo_sb = const.tile([P, C], f32, tag="bo")
            nc.scalar.dma_start(out=bo_sb, in_=bo_bc)
            bpt_sb = const.tile([P, H, KC, SEQ], bf16, tag="bpt")
            for h in range(H):
                # Issued from the ACT queue: 4 MB of bias would otherwise
                # delay the first row's input DMAs behind it on Sync.
                nc.scalar.dma_start(
                    out=bpt_sb[:, h],
                    in_=bpt[h].rearrange("(kc p) q -> p kc q", p=P),
                )

            ident = const.tile([P, P], f32, tag="ident")
            make_identity(nc, ident)
            ident_b = const.tile([P, P], bf16, tag="ident_b")
            nc.vector.tensor_copy(ident_b, ident)
            ones_c = const.tile([P, 1], bf16, tag="ones_c")
            nc.vector.memset(ones_c, 1.0)

            # --- main loop ---------------------------------------------------
            with (
                tc.tile_pool(name="xt", bufs=2) as xt,
                tc.tile_pool(name="pj", bufs=2) as pj,
                tc.tile_pool(name="gp", bufs=1) as gp,
                tc.tile_pool(name="gh", bufs=2) as gh,
                tc.tile_pool(name="vv", bufs=2) as vv,
                tc.tile_pool(name="ee", bufs=4) as ee,
                tc.tile_pool(name="ot", bufs=2) as ot,
                tc.tile_pool(name="dn", bufs=2) as dn,
                tc.tile_pool(name="sa", bufs=2) as sa,
                tc.tile_pool(name="ou", bufs=2) as ou,
                tc.tile_pool(name="psA", bufs=2, space="PSUM") as psA,
                tc.tile_pool(name="psQ", bufs=2, space="PSUM") as psQ,
                tc.tile_pool(name="psO", bufs=2, space="PSUM") as psO,
            ):
                def emit_gate(xTs):
                    gth = gh.tile([P, HC, SEQ], f32, tag="gth")
                    for hc in range(HC):
                        pp = psA.tile([P, SEQ], f32, tag="psA")
                        for cc in range(CC):
                            nc.tensor.matmul(
                                pp,
                                w_sbs["wg"][:, cc, P * hc : P * (hc + 1)],
                                xTs["q"][:, cc, :],
                                start=(cc == 0),
                                stop=(cc == CC - 1),
                            )
                        # sigmoid(x + bg) = 0.5*tanh((x + bg)/2) + 0.5
                        nc.scalar.activation(
                            gth[:, hc, :],
                            pp,
                            Tanh,
                            bias=bgh_sb[:, hc : hc + 1],
                            scale=0.5,
                        )
                    return gth

                def emit_inputs_g(n, defer_gate=False):
                    # A: inputs arrive pre-transposed [C, seq] bf16. The gate
                    # projection + tanh are emitted here — a full head-group
                    # phase ahead of the q/k/v projections — so the tanh sits
                    # early in the ACT queue (before the hg1 exps) and its
                    # psA read never blocks later PSUM reuse.
                    xTs = {}
                    for name, src_ap in (("q", qx), ("k", kx), ("v", vx)):
                        xT = xt.tile([P, CC, SEQ], bf16, tag=f"xt_{name}")
                        nc.sync.dma_start(
                            out=xT,
                            in_=src_ap[n].rearrange("(cc p) s -> p cc s", p=P),
                        )
                        xTs[name] = xT

                    gth = None
                    if not defer_gate:
                        gth = emit_gate(xTs)

                    oT = ot.tile([P, HG, SEQ], f32, tag="oT", name="oT")
                    den_t = dn.tile(
                        [P, H * KC], f32, tag="den_t", name="den_t"
                    )
                    return {
                        "n": n, "xTs": xTs, "gth": gth,
                        "oT": oT, "den_t": den_t, "Es": {},
                    }  # gth is None when defer_gate (row 0)

                def emit_proj_qkv(st):
                    # B: q/k/v projections (bf16 in, fp32 PSUM, bf16 out)
                    xTs = st["xTs"]
                    qT = pj.tile([P, HC, SEQ], bf16, tag="qT")
                    kT = pj.tile([P, HC, SEQ], bf16, tag="kT")
                    for dst, wname, src in (
                        (qT, "wq", xTs["q"]),
                        (kT, "wk", xTs["k"]),
                    ):
                        for hc in range(HC):
                            pp = psA.tile([P, SEQ], f32, tag="psA")
                            for cc in range(CC):
                                nc.tensor.matmul(
                                    pp,
                                    w_sbs[wname][:, cc, P * hc : P * (hc + 1)],
                                    src[:, cc, :],
                                    start=(cc == 0),
                                    stop=(cc == CC - 1),
                                )
                            nc.vector.tensor_copy(dst[:, hc, :], pp)

                    if st["gth"] is None:
                        # Row 0: gate deferred until after q/k so the first
                        # tanh's psA WAR lands on the v-projection, which by
                        # then has slack, instead of stalling q-proj.
                        st["gth"] = emit_gate(xTs)

                    v_sb = vv.tile([P, KC, H, CH + 1], bf16, tag="v")
                    # Lane CH is the ones column that accumulates the softmax
                    # denominator during the AV matmul.
                    nc.vector.tensor_copy(
                        v_sb[:, :, :, CH : CH + 1],
                        ones_c[:, None, None, :].to_broadcast([P, KC, H, 1]),
                    )
                    for rc in range(KC):
                        pp = psA.tile([P, SEQ], f32, tag="psA")
                        for cc in range(CC):
                            nc.tensor.matmul(
                                pp[:, 0:HID],
                                xTs["v"][:, cc, P * rc : P * (rc + 1)],
                                w_sbs["wv"][:, cc, :],
                                start=(cc == 0),
                                stop=(cc == CC - 1),
                            )
                        nc.vector.tensor_copy(
                            v_sb[:, rc, :, 0:CH],
                            pp[:, 0:HID].rearrange("p (h c) -> p h c", h=H),
                        )
                    st["qT"], st["kT"], st["v_sb"] = qT, kT, v_sb

                def emit_av_group(st, hg, h2):
                    # One head's AV accumulation + staging out of PSUM.
                    h = 4 * hg + h2
                    Es = st["Es"][hg]
                    po = psO.tile([CH + 1, SEQ], f32, tag="o")
                    for kc in range(KC):
                        nc.tensor.matmul(
                            po,
                            st["v_sb"][:, kc, h, :],
                            Es[h2 // 2][:, kc, h2 % 2, :],
                            start=(kc == 0),
                            stop=(kc == KC - 1),
                        )
                    stg = ot.tile([CH + 1, SEQ], f32, tag="ostag")
                    nc.vector.tensor_copy(stg, po)
                    nc.sync.dma_start(
                        out=st["oT"][CH * h2 : CH * (h2 + 1), hg, :],
                        in_=stg[0:CH, :],
                    )
                    # den_t[p, 4h + j] = den_h[4p + j]: q-major stream lands
                    # p-major so the DVE reciprocal runs on a [128, 32] tile
                    # (256 cycles, not 4096). Issue alternates between the
                    # ACT queue (also a HW-DGE engine) and Sync to spread the
                    # per-DMA issue cost.
                    den_eng = nc.scalar if h % 2 == 0 else nc.sync
                    den_eng.dma_start(
                        out=st["den_t"][:, 4 * h : 4 * (h + 1)],
                        in_=stg[CH : CH + 1, :],
                    )

                def emit_attn_phase(st, hg, carry, own_av=False):
                    # S^T + bias + exp for head group hg of row st, with the
                    # carried-over head group's AV matmuls interleaved after
                    # every second chunk — they depend only on long-finished
                    # exps, so they fill the PE stalls that occur when a
                    # chunk's PSUM recycle waits on the ACT exp drain.
                    n = st["n"]
                    qT, kT = st["qT"], st["kT"]
                    Es = {}
                    for pr in range(2):
                        Es[pr] = ee.tile(
                            [P, KC, 2, SEQ], bf16, tag="E", name=f"E_{pr}"
                        )
                    st["Es"][hg] = Es
                    slot = 0
                    # pr-outer order (own_av mode, last row only) lets this
                    # head group's own AV matmuls start as soon as each
                    # pair's exps drain, shrinking the epilogue pipe-drain.
                    chunk_order = (
                        [(kc, pr) for pr in range(2) for kc in range(KC)]
                        if own_av
                        else [(kc, pr) for kc in range(KC) for pr in range(2)]
                    )
                    for kc, pr in chunk_order:
                        if True:
                            eng = _bias_engine(hg, pr, kc)
                            sp = psQ.tile(
                                [P, 2 * SEQ], f32, tag="qk", name="qk"
                            )
                            for j in range(2):
                                h2 = 2 * pr + j
                                nc.tensor.matmul(
                                    sp[:, SEQ * j : SEQ * (j + 1)],
                                    kT[
                                        CH * h2 : CH * (h2 + 1),
                                        hg,
                                        P * kc : P * (kc + 1),
                                    ],
                                    qT[CH * h2 : CH * (h2 + 1), hg, :],
                                    start=True,
                                    stop=(eng != "pe"),
                                    tile_position=(CH * h2, 0),
                                )
                            if eng == "pe":
                                # bias_pair added in PSUM via identity
                                # matmuls (PE; one per head — a matmul
                                # write cannot span PSUM banks)
                                for j in range(2):
                                    h = 4 * hg + 2 * pr + j
                                    nc.tensor.matmul(
                                        sp[:, SEQ * j : SEQ * (j + 1)],
                                        ident_b,
                                        bpt_sb[:, h, kc, :],
                                        start=False,
                                        stop=True,
                                    )
                                nc.scalar.activation(
                                    Es[pr][:, kc, :, :],
                                    sp.rearrange("p (h q) -> p h q", h=2),
                                    Exp,
                                    bias=bm_sb[:, kc, n : n + 1],
                                )
                            elif eng == "dve":
                                # bias_pair added on DVE, both heads in one op
                                h = 4 * hg + 2 * pr
                                sadd = sa.tile(
                                    [P, 2, SEQ], f32, tag="sadd", name="sadd"
                                )
                                nc.vector.tensor_add(
                                    sadd,
                                    sp.rearrange("p (h q) -> p h q", h=2),
                                    bpt_sb[:, h : h + 2, kc, :],
                                )
                                nc.scalar.activation(
                                    Es[pr][:, kc, :, :],
                                    sadd,
                                    Exp,
                                    bias=bm_sb[:, kc, n : n + 1],
                                )
                            if not own_av:
                                if slot % 2 == 1 and carry is not None:
                                    emit_av_group(carry[0], carry[1], slot // 2)
                            else:
                                # slots 1,3 -> carry AV 0,1; slot 4 -> own
                                # AV h2=0 (pr0 exps done); slots 5,7 -> carry
                                # AV 2,3; slot 6 -> own AV h2=1
                                if slot in (1, 3, 5, 7) and carry is not None:
                                    emit_av_group(
                                        carry[0], carry[1],
                                        {1: 0, 3: 1, 5: 2, 7: 3}[slot],
                                    )
                                elif slot in (4, 6):
                                    emit_av_group(st, hg, (slot - 4) // 2)
                            slot += 1

                def emit_tail_pre(st, last=False):
                    # D1: normalize + gate chain (DVE/GPSIMD/DMA only). Runs
                    # one row late; emitted early in the next row's stream so
                    # its cross-engine serial chain overlaps the next row's
                    # PE-heavy phases.
                    rden = dn.tile([P, H * KC], f32, tag="rden")
                    nc.vector.reciprocal(rden, st["den_t"])
                    dscr = drp.tile([H, SEQ], f32, tag="dscr")
                    # dscr[h, 4p + j] = rden[p, 4h + j]: p-major stream is
                    # q-contiguous in DRAM.
                    nc.sync.dma_start(
                        out=dscr.rearrange("h (p j) -> p h j", p=P),
                        in_=rden.rearrange("p (h j) -> p h j", h=H),
                    )
                    rbc = gp.tile([P, HG, SEQ], f32, tag="rbc")
                    for h in range(H):
                        # Broadcast DMAs issue from the GPSIMD queue (software
                        # DGE) to relieve the saturated Sync queue; in the
                        # epilogue (last row) Sync is idle, so split across
                        # both queues to halve the serial drain.
                        eng = nc.sync if (last and h % 2 == 0) else nc.gpsimd
                        eng.dma_start(
                            out=rbc[CH * (h % 4) : CH * (h % 4 + 1), h // 4, :],
                            in_=dscr[h : h + 1, :].to_broadcast([CH, SEQ]),
                        )
                    gth = st["gth"]
                    oTg = gp.tile([P, HG, SEQ], bf16, tag="oTg")
                    st["oTg"] = oTg
                    # sigmoid finish in-place into gth (the 0.5 scale is
                    # folded into wo host-side: out = 0.5*((t+1)r*o)@wo),
                    # then fold the reciprocal denominator in-place. The two
                    # head-group halves run on opposite engines (GP handles
                    # hc0's scalar+fold, DVE its final mul, and vice versa)
                    # so the serial chain halves in wall-clock.
                    nc.gpsimd.tensor_scalar(
                        gth[:, 0, :], gth[:, 0, :], 1.0, 1.0, MULT, ADD
                    )
                    nc.vector.tensor_scalar(
                        gth[:, 1, :], gth[:, 1, :], 1.0, 1.0, MULT, ADD
                    )
                    nc.gpsimd.tensor_mul(
                        rbc[:, 0, :], rbc[:, 0, :], gth[:, 0, :]
                    )
                    nc.vector.tensor_mul(
                        rbc[:, 1, :], rbc[:, 1, :], gth[:, 1, :]
                    )
                    nc.vector.tensor_mul(
                        oTg[:, 0, :], st["oT"][:, 0, :], rbc[:, 0, :]
                    )
                    nc.gpsimd.tensor_mul(
                        oTg[:, 1, :], st["oT"][:, 1, :], rbc[:, 1, :]
                    )

                def emit_tail_mm(st):
                    # D2: output projection, emitted after the next row's
                    # attention matmuls so the D1 chain has time to finish.
                    n, oTg = st["n"], st["oTg"]
                    osb = ou.tile([P, QC, C], f32, tag="osb")
                    for qc in range(QC):
                        pp = psA.tile([P, SEQ], f32, tag="psA")
                        for hc in range(HC):
                            nc.tensor.matmul(
                                pp[:, 0:C],
                                oTg[:, hc, P * qc : P * (qc + 1)],
                                wo_sb[:, hc, :],
                                start=(hc == 0),
                                stop=(hc == HC - 1),
                            )
                        nc.vector.tensor_add(osb[:, qc, :], pp[:, 0:C], bo_sb)
                    nc.sync.dma_start(
                        out=out[n].rearrange("(qc p) c -> p qc c", p=P),
                        in_=osb,
                    )

                def emit_tail_last(st):
                    # Epilogue for the final row, pipelined by head-group
                    # half: the heads-0-3 chain overlaps the heads-6-7 AV
                    # flush, and the first half of the output projection
                    # starts before the heads-4-7 chain finishes.
                    n = st["n"]
                    gth = st["gth"]
                    oTg = gp.tile([P, HG, SEQ], bf16, tag="oTg", name="oTgl")
                    st["oTg"] = oTg
                    rbc = gp.tile([P, HG, SEQ], f32, tag="rbc", name="rbcl")
                    rden0 = dn.tile([P, 16], f32, tag="rden0", name="rden0")
                    nc.vector.reciprocal(rden0, st["den_t"][:, 0:16])
                    dscr0 = drp.tile([4, SEQ], f32, tag="dscr0", name="dscr0")
                    nc.sync.dma_start(
                        out=dscr0.rearrange("h (p j) -> p h j", p=P),
                        in_=rden0.rearrange("p (h j) -> p h j", h=4),
                    )
                    for h in range(4):
                        eng = nc.sync if h % 2 == 0 else nc.gpsimd
                        eng.dma_start(
                            out=rbc[CH * h : CH * (h + 1), 0, :],
                            in_=dscr0[h : h + 1, :].to_broadcast([CH, SEQ]),
                        )
                    HS = SEQ // 2
                    nc.gpsimd.tensor_scalar(
                        gth[:, 0, :], gth[:, 0, :], 1.0, 1.0, MULT, ADD
                    )
                    # fold + gate-mul run half-width on both engines — this
                    # chain is raw epilogue latency, nothing hides it
                    nc.gpsimd.tensor_mul(
                        rbc[:, 0, 0:HS], rbc[:, 0, 0:HS], gth[:, 0, 0:HS]
                    )
                    nc.vector.tensor_mul(
                        rbc[:, 0, HS:SEQ], rbc[:, 0, HS:SEQ],
                        gth[:, 0, HS:SEQ],
                    )
                    nc.vector.tensor_mul(
                        oTg[:, 0, 0:HS], st["oT"][:, 0, 0:HS],
                        rbc[:, 0, 0:HS],
                    )
                    nc.gpsimd.tensor_mul(
                        oTg[:, 0, HS:SEQ], st["oT"][:, 0, HS:SEQ],
                        rbc[:, 0, HS:SEQ],
                    )
                    for h2 in (2, 3):
                        emit_av_group(st, 1, h2)
                    osb = ou.tile([P, QC, C], f32, tag="osb", name="osbl")
                    pps = {}
                    for qc in (0, 1):
                        pp = psA.tile(
                            [P, SEQ], f32, tag="psA", name=f"psAl{qc}"
                        )
                        nc.tensor.matmul(
                            pp[:, 0:C],
                            oTg[:, 0, P * qc : P * (qc + 1)],
                            wo_sb[:, 0, :],
                            start=True,
                            stop=False,
                        )
                        pps[qc] = pp
                    rden1 = dn.tile([P, 16], f32, tag="rden1", name="rden1")
                    nc.vector.reciprocal(rden1, st["den_t"][:, 16:32])
                    dscr1 = drp.tile([4, SEQ], f32, tag="dscr1", name="dscr1")
                    nc.sync.dma_start(
                        out=dscr1.rearrange("h (p j) -> p h j", p=P),
                        in_=rden1.rearrange("p (h j) -> p h j", h=4),
                    )
                    for h in range(4):
                        eng = nc.sync if h % 2 == 0 else nc.gpsimd
                        eng.dma_start(
                            out=rbc[CH * h : CH * (h + 1), 1, :],
                            in_=dscr1[h : h + 1, :].to_broadcast([CH, SEQ]),
                        )
                    nc.vector.tensor_scalar(
                        gth[:, 1, :], gth[:, 1, :], 1.0, 1.0, MULT, ADD
                    )
                    nc.vector.tensor_mul(
                        rbc[:, 1, 0:HS], rbc[:, 1, 0:HS], gth[:, 1, 0:HS]
                    )
                    nc.gpsimd.tensor_mul(
                        rbc[:, 1, HS:SEQ], rbc[:, 1, HS:SEQ],
                        gth[:, 1, HS:SEQ],
                    )
                    nc.gpsimd.tensor_mul(
                        oTg[:, 1, 0:HS], st["oT"][:, 1, 0:HS],
                        rbc[:, 1, 0:HS],
                    )
                    nc.vector.tensor_mul(
                        oTg[:, 1, HS:SEQ], st["oT"][:, 1, HS:SEQ],
                        rbc[:, 1, HS:SEQ],
                    )
                    for qc in (0, 1):
                        nc.tensor.matmul(
                            pps[qc][:, 0:C],
                            oTg[:, 1, P * qc : P * (qc + 1)],
                            wo_sb[:, 1, :],
                            start=False,
                            stop=True,
                        )
                        nc.vector.tensor_add(
                            osb[:, qc, :], pps[qc][:, 0:C], bo_sb
                        )
                    for qc in (2, 3):
                        pp = psA.tile(
                            [P, SEQ], f32, tag="psA", name=f"psAl{qc}"
                        )
                        for hc in range(HC):
                            nc.tensor.matmul(
                                pp[:, 0:C],
                                oTg[:, hc, P * qc : P * (qc + 1)],
                                wo_sb[:, hc, :],
                                start=(hc == 0),
                                stop=(hc == HC - 1),
                            )
                        nc.vector.tensor_add(
                            osb[:, qc, :], pp[:, 0:C], bo_sb
                        )
                    nc.sync.dma_start(
                        out=out[n].rearrange("(qc p) c -> p qc c", p=P),
                        in_=osb,
                    )

                prev = None
                cur = emit_inputs_g(0, defer_gate=True)
                emit_proj_qkv(cur)
                for n in range(NL):
                    # S(n, hg0), interleaved with AV of prev row's hg1
                    emit_attn_phase(
                        cur, 0, (prev, 1) if prev is not None else None
                    )
                    if prev is not None:
                        emit_tail_pre(prev)
                    nxt = emit_inputs_g(n + 1) if n + 1 < NL else None
                    # S(n, hg1), interleaved with AV of this row's hg0
                    emit_attn_phase(cur, 1, (cur, 0), own_av=(nxt is None))
                    if prev is not None:
                        emit_tail_mm(prev)
                    if nxt is not None:
                        emit_proj_qkv(nxt)
                    prev, cur = cur, nxt
                emit_tail_last(prev)

    return nc


_NC_CACHE = None


def _get_nc():
    global _NC_CACHE
    if _NC_CACHE is None:
        _NC_CACHE = _build_nc()
    return _NC_CACHE


def _prepare_in_maps(q_x, k_x, v_x, bias_mask, bias_pair, wq, wk, wv, wg, bg, wo, bo):
    wq_s = (wq / math.sqrt(CH)).astype(np_bf16)
    bpt_f = np.ascontiguousarray(
        np.transpose(bias_pair[0, 0], (0, 2, 1)), dtype=np.float32
    )  # [h, k, q]
    bpt = bpt_f.astype(np_bf16)
    bgh = np.ascontiguousarray((bg / 2.0).reshape(HC, P).T, dtype=np.float32)
    bo_bc = np.ascontiguousarray(np.tile(bo[None, :], (P, 1)), dtype=np.float32)
    bm_all = np.asarray(bias_mask[0, :, 0, 0, :], dtype=np.float32)  # [64, 512]

    in_maps = []
    for c in range(N_CORES):
        ns = slice(NL * c, NL * (c + 1))
        bm_r = np.ascontiguousarray(
            bm_all[ns].reshape(NL, KC, P).transpose(2, 1, 0), dtype=np.float32
        )
        in_maps.append(
            {
                "qx": np.ascontiguousarray(
                    q_x[0, ns].transpose(0, 2, 1)
                ).astype(np_bf16),
                "kx": np.ascontiguousarray(
                    k_x[0, ns].transpose(0, 2, 1)
                ).astype(np_bf16),
                "vx": np.ascontiguousarray(
                    v_x[0, ns].transpose(0, 2, 1)
                ).astype(np_bf16),
                "bpt": bpt,
                "bm": bm_r,
                "wq": wq_s,
                "wk": np.asarray(wk).astype(np_bf16),
                "wv": np.asarray(wv).astype(np_bf16),
                "wg": np.asarray(wg).astype(np_bf16),
                "bgh": bgh,
                # gate = (tanh((g+bg)/2) + 1) * r; the missing 0.5 of the
                # sigmoid identity is folded in here.
                "wo": (np.asarray(wo) * 0.5).astype(np_bf16),
                "bo_bc": bo_bc,
            }
        )
    return in_maps


def run(trace=False, **inputs):
    """Run the kernel; returns (output, BassKernelResults)."""
    args = {k: np.asarray(v) for k, v in inputs.items()}
    in_maps = _prepare_in_maps(
        args["q_x"], args["k_x"], args["v_x"], args["bias_mask"],
        args["bias_pair"], args["wq"], args["wk"], args["wv"], args["wg"],
        args["bg"], args["wo"], args["bo"],
    )
    nc = _get_nc()
    res = run_bass_kernel_spmd(nc, in_maps, list(range(N_CORES)), trace=trace)
    out = np.empty((1, NL * N_CORES, SEQ, C), dtype=np.float32)
    for c in range(N_CORES):
        out[0, NL * c : NL * (c + 1)] = res.results[c]["out"]
    return out, res


def kernel(**inputs):
    out, _ = run(trace=False, **inputs)
    return out


if __name__ == "__main__":
    rng = np.random.default_rng(0)
    demo = {
        "q_x": rng.standard_normal((1, 64, SEQ, C)).astype(np.float32),
        "k_x": rng.standard_normal((1, 64, SEQ, C)).astype(np.float32),
        "v_x": rng.standard_normal((1, 64, SEQ, C)).astype(np.float32),
        "bias_mask": rng.standard_normal((1, 64, 1, 1, SEQ)).astype(np.float32),
        "bias_pair": rng.standard_normal((1, 1, H, SEQ, SEQ)).astype(np.float32),
        "wq": (rng.standard_normal((C, HID)) / 16).astype(np.float32),
        "wk": (rng.standard_normal((C, HID)) / 16).astype(np.float32),
        "wv": (rng.standard_normal((C, HID)) / 16).astype(np.float32),
        "wg": (rng.standard_normal((C, HID)) * 0.02).astype(np.float32),
        "bg": np.ones((HID,), dtype=np.float32),
        "wo": (rng.standard_normal((HID, C)) * 0.02).astype(np.float32),
        "bo": np.zeros((C,), dtype=np.float32),
    }
    o = kernel(**demo)
    print("kernel ran, out shape", o.shape, "mean", float(np.abs(o).mean()))
